# revision 13
# baseline (speedup 1.0000x reference)
"""ComplexMultiheadAttention Trainium2 kernel.

Sharding: 8 cores = 2 batches x 4 head-pairs. Each core computes the full
4-way complex-combination attention for its 2 heads on its batch, plus the
partial output projection over its 64 INNER columns; host sums the 4 partial
y's per batch.

Device layout (per core) is fully "transposed": q/k in [d-on-partitions,
tokens-free] so scores S^T = K @ Q^T come out with k-tokens on partitions and
softmax's sum reduction folds into the attention*V matmul via a ones column
in V (no on-device transposes anywhere).

  rowgroup g of the 128-row Q-stack:  [qr_h0 | qi_h0 | qr_h1 | qi_h1]
  K-stack A: [kr_h0 | -ki_h0 | kr_h1 | -ki_h1]  -> combos rr, ii
  K-stack B: [-ki_h0 | kr_h0 | -ki_h1 | kr_h1]  -> combos ri, ir

Scores skip max-subtraction (|s|<~8 with these input scales, exp is safe in
fp32/bf16). Output projection contracts all 8 den-scaled O^T blocks against
host-stacked signed Wo so o_r = rr-ii and o_i = ri+ir never materialize.

Perf structure: everything bf16 on the PE (one LDW cadence, no fp32r), 512
query columns per tile so PSUM holds sc x3 (2 banks) + av x2 (1 bank), and a
one-deep software pipeline [S(i+1) | exp(i) on ACT | AV(i)] that keeps the
tensor engine busy past the HAM clock-gate window (sustained 2.4 GHz). One
1024-wide exp per iteration covers both combos of the S tile; the den
reciprocal uses the fast single-pass DVE approximation off the critical path
with av double-buffered.
"""

import numpy as np
import ml_dtypes

B, N, DIM = 2, 2048, 256
HEADS, DHEAD = 8, 32
INNER = HEADS * DHEAD
NCORES = 8
SCALE = DHEAD**-0.5
KT = 5  # contraction tiles for qkv projection: 2x xr^T + 2x xi^T + ones-row tile
QW = 512  # query-slice width
NQS = N // QW
NKT = N // 128

_BF16 = ml_dtypes.bfloat16

_cache = {}


def _build():
    import concourse.bacc as bacc
    import concourse.bass as bass
    import concourse.tile as tile
    from concourse import mybir

    f32 = mybir.dt.float32
    bf16 = mybir.dt.bfloat16

    nc = bacc.Bacc("TRN2", target_bir_lowering=False, debug=True)

    xt = nc.dram_tensor("xt", [KT, 128, N], bf16, kind="ExternalInput")
    wq = nc.dram_tensor("wq", [KT, 128, 128], bf16, kind="ExternalInput")
    wka = nc.dram_tensor("wka", [KT, 128, 128], bf16, kind="ExternalInput")
    wkb = nc.dram_tensor("wkb", [KT, 128, 128], bf16, kind="ExternalInput")
    wv = nc.dram_tensor("wv", [KT, 128, 132], bf16, kind="ExternalInput")
    wpr = nc.dram_tensor("wpr", [2, 128, 256], bf16, kind="ExternalInput")
    wpi = nc.dram_tensor("wpi", [2, 128, 256], bf16, kind="ExternalInput")
    yr = nc.dram_tensor("yr", [N, DIM], f32, kind="ExternalOutput")
    yi = nc.dram_tensor("yi", [N, DIM], f32, kind="ExternalOutput")

    with tile.TileContext(nc) as tc:
        from contextlib import ExitStack

        with ExitStack() as ctx:
            singles = ctx.enter_context(tc.tile_pool(name="singles", bufs=1))
            sc_ps = ctx.enter_context(tc.tile_pool(name="sc_ps", bufs=2, space="PSUM"))
            av_ps = ctx.enter_context(tc.tile_pool(name="av_ps", bufs=2, space="PSUM"))
            yp_ps = ctx.enter_context(tc.tile_pool(name="yp_ps", bufs=2, space="PSUM"))
            e_pool = ctx.enter_context(tc.tile_pool(name="e_pool", bufs=4))
            so_pool = ctx.enter_context(tc.tile_pool(name="so_pool", bufs=4))
            rc_pool = ctx.enter_context(tc.tile_pool(name="rc_pool", bufs=4))
            bc_pool = ctx.enter_context(tc.tile_pool(name="bc_pool", bufs=4))
            y_pool = ctx.enter_context(tc.tile_pool(name="y_pool", bufs=4))

            # ---- load inputs ----
            sb_x = singles.tile([128, KT, N], bf16)
            sb_wq = singles.tile([128, KT, 128], bf16)
            sb_wka = singles.tile([128, KT, 128], bf16)
            sb_wkb = singles.tile([128, KT, 128], bf16)
            sb_wv = singles.tile([128, KT, 132], bf16)
            sb_wpr = singles.tile([128, 2, 256], bf16)
            sb_wpi = singles.tile([128, 2, 256], bf16)
            for kt in range(KT):
                nc.sync.dma_start(sb_x[:, kt, :], xt[kt])
                nc.sync.dma_start(sb_wq[:, kt, :], wq[kt])
                nc.sync.dma_start(sb_wka[:, kt, :], wka[kt])
                nc.sync.dma_start(sb_wkb[:, kt, :], wkb[kt])
                nc.sync.dma_start(sb_wv[:, kt, :], wv[kt])
            for j in range(2):
                nc.sync.dma_start(sb_wpr[:, j, :], wpr[j])
                nc.sync.dma_start(sb_wpi[:, j, :], wpi[j])

            # ---- phase 1: projections ----
            # Q^T / KA^T / KB^T: [128 stacked d-rows, 2048 tokens] bf16
            qt = singles.tile([128, N], bf16)
            kat = singles.tile([128, N], bf16)
            kbt = singles.tile([128, N], bf16)

            def emit_proj(dst, w):
                for s in range(4):  # 512-token slices
                    ps = sc_ps.tile([128, 1024], f32, tag="sc")
                    for kt in range(KT):
                        nc.tensor.matmul(
                            ps[:, 0:512],
                            w[:, kt, :],
                            sb_x[:, kt, s * 512 : (s + 1) * 512],
                            start=(kt == 0),
                            stop=(kt == KT - 1),
                        )
                    nc.vector.tensor_copy(dst[:, s * 512 : (s + 1) * 512], ps[:, 0:512])

            # V (+ones cols): [128 tokens, 132] per token tile, bf16
            v_sb = singles.tile([128, NKT, 132], bf16)

            def emit_v():
                for mt in range(NKT):
                    ps = sc_ps.tile([128, 1024], f32, tag="sc")
                    for kt in range(KT):
                        nc.tensor.matmul(
                            ps[:, 0:132],
                            sb_x[:, kt, mt * 128 : (mt + 1) * 128],
                            sb_wv[:, kt, :],
                            start=(kt == 0),
                            stop=(kt == KT - 1),
                        )
                    nc.vector.tensor_copy(v_sb[:, mt, :], ps[:, 0:132])

            emit_proj(qt, sb_wq)
            emit_proj(kat, sb_wka)

            # ---- phase 2: software-pipelined attention ----
            # groups ordered A-pairing first so kbt can be computed during
            # the first A groups
            iters = [
                (qs, pairing, half, kt)
                for qs in range(NQS)
                for pairing in range(2)
                for half in range(2)
                for kt in range(NKT)
            ]

            av_tiles = {}
            so_tiles = {}
            es_tiles = {}

            def emit_s_exp(it):
                qs, pairing, half, kt = it
                ksel = kat if pairing == 0 else kbt
                sc = sc_ps.tile([128, 1024], f32, tag="sc")
                for i in range(2):  # combo within the (pairing, half) group
                    g = 2 * half + i
                    nc.tensor.matmul(
                        sc[:, i * 512 : (i + 1) * 512],
                        ksel[32 * g : 32 * (g + 1), kt * 128 : (kt + 1) * 128],
                        qt[32 * g : 32 * (g + 1), qs * QW : (qs + 1) * QW],
                        start=True,
                        stop=True,
                        tile_position=(32 * g, 0),
                    )
                es = e_pool.tile([128, 1024], bf16, tag="e")
                nc.scalar.activation(
                    es[:], sc[:], mybir.ActivationFunctionType.Exp, scale=SCALE
                )
                es_tiles[it] = es

            def emit_av(it):
                qs, pairing, half, kt = it
                es = es_tiles.pop(it)
                if kt == 0:
                    av_tiles[(qs, pairing, half)] = av_ps.tile(
                        [128, QW], f32, tag="av", name="av"
                    )
                av = av_tiles[(qs, pairing, half)]
                first, last = kt == 0, kt == NKT - 1
                vcol = [(2 * half) * 33, (2 * half + 1) * 33]
                # combo0 -> psum rows 0:33, combo1 -> rows 64:97 (den in 32/96);
                # disjoint column strips run concurrently on the PE
                nc.tensor.matmul(
                    av[0:33, :],
                    v_sb[:, kt, vcol[0] : vcol[0] + 33],
                    es[:, 0:512],
                    start=first,
                    stop=last,
                    tile_position=(0, 0),
                )
                nc.tensor.matmul(
                    av[64:97, :],
                    v_sb[:, kt, vcol[1] : vcol[1] + 33],
                    es[:, 512:1024],
                    start=first,
                    stop=last,
                    tile_position=(0, 64),
                )

            def emit_den_scale(qs, pairing, half):
                av = av_tiles.pop((qs, pairing, half))
                if (qs, pairing) not in so_tiles:
                    so_tiles[(qs, pairing)] = so_pool.tile(
                        [128, QW], bf16, tag="so", name="so"
                    )
                so = so_tiles[(qs, pairing)]
                # so rows [c0_h0 | c0_h1 | c1_h0 | c1_h1]
                # both reciprocals first: the DVE is strict FIFO, so a mul
                # that waits on the gpsimd broadcast must not sit ahead of
                # the second (3.4us) reciprocal
                recs, bcs = [], []
                for p0 in (0, 64):
                    rec = rc_pool.tile([1, QW], f32, tag="rc")
                    nc.vector.reciprocal(rec[:], av[p0 + 32 : p0 + 33, :])
                    recs.append(rec)
                for rec in recs:
                    bc = bc_pool.tile([32, QW], f32, tag="bc")
                    nc.gpsimd.partition_broadcast(bc[:], rec[:], channels=32)
                    bcs.append(bc)
                for p0, dst0, bc in ((0, 0, bcs[0]), (64, 64, bcs[1])):
                    nc.vector.tensor_mul(
                        so[dst0 + 32 * half : dst0 + 32 * half + 32, :],
                        av[p0 : p0 + 32, :],
                        bc[:],
                    )

            def emit_projection(qs):
                so_a = so_tiles.pop((qs, 0))
                so_b = so_tiles.pop((qs, 1))
                for mt in range(QW // 128):
                    tok = qs * QW + mt * 128
                    for wsel, ydram in ((sb_wpr, yr), (sb_wpi, yi)):
                        yps = yp_ps.tile([128, 256], f32, tag="yp")
                        nc.tensor.matmul(
                            yps[:],
                            so_a[:, mt * 128 : (mt + 1) * 128],
                            wsel[:, 0, :],
                            start=True,
                            stop=False,
                        )
                        nc.tensor.matmul(
                            yps[:],
                            so_b[:, mt * 128 : (mt + 1) * 128],
                            wsel[:, 1, :],
                            start=False,
                            stop=True,
                        )
                        ysb = y_pool.tile([128, 256], f32, tag="y")
                        nc.vector.tensor_copy(ysb[:], yps[:])
                        nc.sync.dma_start(ydram[tok : tok + 128, :], ysb[:])

            # lookahead-2 pipeline so the PE never waits on ACT within an
            # iteration; projections are emitted PROJ_DELAY iterations after
            # their den-scale so the PE-side projection matmuls don't stall
            # on the DVE reciprocal chain (stalls >3.4us re-throttle HAM)
            PROJ_DELAY = 14
            pending_proj = []
            for i in range(-2, len(iters)):
                if i + 2 < len(iters):
                    emit_s_exp(iters[i + 2])
                if i == -2:
                    continue
                if i == -1:
                    # V and kbt land on the PE behind the first S matmuls so
                    # the first exp starts ~18us earlier; V feeds AV(0) which
                    # is emitted after it, kbt only the B pairing (iter 32+)
                    emit_v()
                    emit_proj(kbt, sb_wkb)
                    continue
                while pending_proj and pending_proj[0][0] <= i:
                    emit_projection(pending_proj.pop(0)[1])
                emit_av(iters[i])
                qs, pairing, half, kt = iters[i]
                if kt == NKT - 1:
                    emit_den_scale(qs, pairing, half)
                    if pairing == 1 and half == 1:
                        pending_proj.append((i + PROJ_DELAY, qs))
            while pending_proj:
                emit_projection(pending_proj.pop(0)[1])

    nc.compile()
    return nc


def _wstack_qk(top, bot, order):
    """[640, 32*len(order)] contraction stack: rows 0:256 coeff of xr,
    256:512 coeff of xi, 512+ zero. order entries: (head, kind)."""
    out = np.zeros((KT * 128, 32 * len(order)), np.float32)
    for g, (h, kind) in enumerate(order):
        rows = slice(32 * h, 32 * h + 32)
        if kind == "r":  # real part: xr@Wr^T - xi@Wi^T
            t, bo = top[rows], -bot[rows]
        elif kind == "i":  # imag part: xr@Wi^T + xi@Wr^T
            t, bo = bot[rows], top[rows]
        else:  # 'ni': negated imag
            t, bo = -bot[rows], -top[rows]
        out[0:256, 32 * g : 32 * (g + 1)] = t.T
        out[256:512, 32 * g : 32 * (g + 1)] = bo.T
    return out


def _prep_core(inp, c):
    b, hp = divmod(c, 4)
    h0, h1 = 2 * hp, 2 * hp + 1

    xt = np.zeros((KT * 128, N), np.float32)
    xt[0:256] = inp["x_r"][b].T
    xt[256:512] = inp["x_i"][b].T
    xt[512] = 1.0

    wq = _wstack_qk(inp["Wq_r"], inp["Wq_i"], [(h0, "r"), (h0, "i"), (h1, "r"), (h1, "i")])
    kr_top, kr_bot = inp["Wkv_r"][:INNER], inp["Wkv_i"][:INNER]
    wka = _wstack_qk(kr_top, kr_bot, [(h0, "r"), (h0, "ni"), (h1, "r"), (h1, "ni")])
    wkb = _wstack_qk(kr_top, kr_bot, [(h0, "ni"), (h0, "r"), (h1, "ni"), (h1, "r")])

    vv = _wstack_qk(
        inp["Wkv_r"][INNER:], inp["Wkv_i"][INNER:],
        [(h0, "r"), (h0, "i"), (h1, "r"), (h1, "i")],
    )  # [640, 128]
    wv = np.zeros((KT * 128, 132), np.float32)
    for blk in range(4):
        wv[:, 33 * blk : 33 * blk + 32] = vv[:, 32 * blk : 32 * (blk + 1)]
        wv[512, 33 * blk + 32] = 1.0  # ones column via the ones-row of x_aug

    # projection stacks: rows of scaledO_A = [rr_h0 | rr_h1 | ii_h0 | ii_h1],
    # scaledO_B = [ri_h0 | ri_h1 | ir_h0 | ir_h1]
    wor = inp["Wo_r"]
    woi = inp["Wo_i"]

    def ocols(h):
        return slice(32 * h, 32 * h + 32)

    wpr = np.zeros((2, 128, 256), np.float32)
    wpi = np.zeros((2, 128, 256), np.float32)
    for j, h in ((0, h0), (1, h1)):
        wpr[0, 32 * j : 32 * j + 32] = wor[:, ocols(h)].T
        wpr[0, 64 + 32 * j : 96 + 32 * j] = -wor[:, ocols(h)].T
        wpr[1, 32 * j : 32 * j + 32] = -woi[:, ocols(h)].T
        wpr[1, 64 + 32 * j : 96 + 32 * j] = -woi[:, ocols(h)].T
        wpi[0, 32 * j : 32 * j + 32] = woi[:, ocols(h)].T
        wpi[0, 64 + 32 * j : 96 + 32 * j] = -woi[:, ocols(h)].T
        wpi[1, 32 * j : 32 * j + 32] = wor[:, ocols(h)].T
        wpi[1, 64 + 32 * j : 96 + 32 * j] = wor[:, ocols(h)].T

    return {
        "xt": xt.reshape(KT, 128, N).astype(_BF16),
        "wq": wq.reshape(KT, 128, 128).astype(_BF16),
        "wka": wka.reshape(KT, 128, 128).astype(_BF16),
        "wkb": wkb.reshape(KT, 128, 128).astype(_BF16),
        "wv": wv.reshape(KT, 128, 132).astype(_BF16),
        "wpr": wpr.astype(_BF16),
        "wpi": wpi.astype(_BF16),
    }


TRACE = False
TRACE_DIR = None
LAST_RESULT = None


def kernel(**inputs):
    global LAST_RESULT
    from concourse.bass_utils import run_bass_kernel_spmd

    inputs = {k: np.asarray(v, dtype=np.float32) for k, v in inputs.items()}
    if "nc" not in _cache:
        _cache["nc"] = _build()
    nc = _cache["nc"]

    in_maps = [_prep_core(inputs, c) for c in range(NCORES)]
    kw = {}
    if TRACE:
        kw = {"trace": True, "tmpdir": TRACE_DIR}
    res = run_bass_kernel_spmd(nc, in_maps, list(range(NCORES)), **kw)
    LAST_RESULT = res

    out = np.zeros((B, N, DIM, 2), np.float32)
    for c in range(NCORES):
        b = c // 4
        out[b, :, :, 0] += res.results[c]["yr"]
        out[b, :, :, 1] += res.results[c]["yi"]
    return out


# revision 14
# speedup vs baseline: 1.0994x; 1.0994x over previous
"""ComplexMultiheadAttention Trainium2 kernel.

Sharding: 8 cores = 2 batches x 4 head-pairs. Each core computes the full
4-way complex-combination attention for its 2 heads on its batch, plus the
partial output projection over its 64 INNER columns; host sums the 4 partial
y's per batch.

Device layout (per core) is fully "transposed": q/k in [d-on-partitions,
tokens-free] so scores S^T = K @ Q^T come out with k-tokens on partitions and
softmax's sum reduction folds into the attention*V matmul via a ones column
in V (no on-device transposes anywhere).

  rowgroup g of the 128-row Q-stack:  [qr_h0 | qi_h0 | qr_h1 | qi_h1]
  K-stack A: [kr_h0 | -ki_h0 | kr_h1 | -ki_h1]  -> combos rr, ii
  K-stack B: [-ki_h0 | kr_h0 | -ki_h1 | kr_h1]  -> combos ri, ir

Scores skip max-subtraction (|s|<~8 with these input scales, exp is safe in
fp32/bf16). Output projection contracts all 8 den-scaled O^T blocks against
host-stacked signed Wo so o_r = rr-ii and o_i = ri+ir never materialize.

Perf structure: everything bf16 on the PE (one LDW cadence, no fp32r), 512
query columns per tile so PSUM holds sc x3 (2 banks) + av x2 (1 bank), and a
one-deep software pipeline [S(i+1) | exp(i) on ACT | AV(i)] that keeps the
tensor engine busy past the HAM clock-gate window (sustained 2.4 GHz). One
1024-wide exp per iteration covers both combos of the S tile; the den
reciprocal uses the fast single-pass DVE approximation off the critical path
with av double-buffered.
"""

import numpy as np
import ml_dtypes

B, N, DIM = 2, 2048, 256
HEADS, DHEAD = 8, 32
INNER = HEADS * DHEAD
NCORES = 8
SCALE = DHEAD**-0.5
KT = 5  # contraction tiles for qkv projection: 2x xr^T + 2x xi^T + ones-row tile
QW = 512  # query-slice width
NQS = N // QW
NKT = N // 128

_BF16 = ml_dtypes.bfloat16

_cache = {}


def _build():
    import concourse.bacc as bacc
    import concourse.bass as bass
    import concourse.tile as tile
    from concourse import mybir

    f32 = mybir.dt.float32
    bf16 = mybir.dt.bfloat16

    nc = bacc.Bacc("TRN2", target_bir_lowering=False, debug=True)

    xt = nc.dram_tensor("xt", [KT, 128, N], bf16, kind="ExternalInput")
    wq = nc.dram_tensor("wq", [KT, 128, 128], bf16, kind="ExternalInput")
    wka = nc.dram_tensor("wka", [KT, 128, 128], bf16, kind="ExternalInput")
    wkb = nc.dram_tensor("wkb", [KT, 128, 128], bf16, kind="ExternalInput")
    wv = nc.dram_tensor("wv", [KT, 128, 132], bf16, kind="ExternalInput")
    wpr = nc.dram_tensor("wpr", [2, 128, 256], bf16, kind="ExternalInput")
    wpi = nc.dram_tensor("wpi", [2, 128, 256], bf16, kind="ExternalInput")
    yr = nc.dram_tensor("yr", [N, DIM], f32, kind="ExternalOutput")
    yi = nc.dram_tensor("yi", [N, DIM], f32, kind="ExternalOutput")

    with tile.TileContext(nc) as tc:
        from contextlib import ExitStack

        with ExitStack() as ctx:
            singles = ctx.enter_context(tc.tile_pool(name="singles", bufs=1))
            sc_ps = ctx.enter_context(tc.tile_pool(name="sc_ps", bufs=3, space="PSUM"))
            av_ps = ctx.enter_context(tc.tile_pool(name="av_ps", bufs=2, space="PSUM"))
            e_pool = ctx.enter_context(tc.tile_pool(name="e_pool", bufs=4))
            so_pool = ctx.enter_context(tc.tile_pool(name="so_pool", bufs=4))
            rc_pool = ctx.enter_context(tc.tile_pool(name="rc_pool", bufs=4))
            bc_pool = ctx.enter_context(tc.tile_pool(name="bc_pool", bufs=4))
            y_pool = ctx.enter_context(tc.tile_pool(name="y_pool", bufs=4))

            # ---- load inputs ----
            sb_x = singles.tile([128, KT, N], bf16)
            sb_wq = singles.tile([128, KT, 128], bf16)
            sb_wka = singles.tile([128, KT, 128], bf16)
            sb_wkb = singles.tile([128, KT, 128], bf16)
            sb_wv = singles.tile([128, KT, 132], bf16)
            sb_wpr = singles.tile([128, 2, 256], bf16)
            sb_wpi = singles.tile([128, 2, 256], bf16)
            for kt in range(KT):
                nc.sync.dma_start(sb_x[:, kt, :], xt[kt])
                nc.sync.dma_start(sb_wq[:, kt, :], wq[kt])
                nc.sync.dma_start(sb_wka[:, kt, :], wka[kt])
                nc.sync.dma_start(sb_wkb[:, kt, :], wkb[kt])
                nc.sync.dma_start(sb_wv[:, kt, :], wv[kt])
            for j in range(2):
                nc.sync.dma_start(sb_wpr[:, j, :], wpr[j])
                nc.sync.dma_start(sb_wpi[:, j, :], wpi[j])

            # ---- phase 1: projections ----
            # Q^T / KA^T / KB^T: [128 stacked d-rows, 2048 tokens] bf16
            qt = singles.tile([128, N], bf16)
            kat = singles.tile([128, N], bf16)
            kbt = singles.tile([128, N], bf16)

            def emit_proj(dst, w):
                for s in range(4):  # 512-token slices
                    ps = sc_ps.tile([128, 1024], f32, tag="sc")
                    for kt in range(KT):
                        nc.tensor.matmul(
                            ps[:, 0:512],
                            w[:, kt, :],
                            sb_x[:, kt, s * 512 : (s + 1) * 512],
                            start=(kt == 0),
                            stop=(kt == KT - 1),
                        )
                    nc.vector.tensor_copy(dst[:, s * 512 : (s + 1) * 512], ps[:, 0:512])

            # V (+ones cols): [128 tokens, 132] per token tile, bf16
            v_sb = singles.tile([128, NKT, 132], bf16)

            def emit_v():
                for mt in range(NKT):
                    ps = sc_ps.tile([128, 1024], f32, tag="sc")
                    for kt in range(KT):
                        nc.tensor.matmul(
                            ps[:, 0:132],
                            sb_x[:, kt, mt * 128 : (mt + 1) * 128],
                            sb_wv[:, kt, :],
                            start=(kt == 0),
                            stop=(kt == KT - 1),
                        )
                    nc.vector.tensor_copy(v_sb[:, mt, :], ps[:, 0:132])

            emit_proj(qt, sb_wq)
            emit_proj(kat, sb_wka)

            # ---- phase 2: software-pipelined attention ----
            # groups ordered A-pairing first so kbt can be computed during
            # the first A groups
            iters = [
                (qs, pairing, half, kt)
                for qs in range(NQS)
                for pairing in range(2)
                for half in range(2)
                for kt in range(NKT)
            ]

            av_tiles = {}
            so_tiles = {}
            es_tiles = {}

            def emit_s_exp(it):
                qs, pairing, half, kt = it
                ksel = kat if pairing == 0 else kbt
                sc = sc_ps.tile([128, 1024], f32, tag="sc")
                for i in range(2):  # combo within the (pairing, half) group
                    g = 2 * half + i
                    nc.tensor.matmul(
                        sc[:, i * 512 : (i + 1) * 512],
                        ksel[32 * g : 32 * (g + 1), kt * 128 : (kt + 1) * 128],
                        qt[32 * g : 32 * (g + 1), qs * QW : (qs + 1) * QW],
                        start=True,
                        stop=True,
                        tile_position=(32 * g, 0),
                    )
                es = e_pool.tile([128, 1024], bf16, tag="e")
                nc.scalar.activation(
                    es[:], sc[:], mybir.ActivationFunctionType.Exp, scale=SCALE
                )
                es_tiles[it] = es

            def emit_av(it):
                qs, pairing, half, kt = it
                es = es_tiles.pop(it)
                if kt == 0:
                    av_tiles[(qs, pairing, half)] = av_ps.tile(
                        [128, QW], f32, tag="av", name="av"
                    )
                av = av_tiles[(qs, pairing, half)]
                first, last = kt == 0, kt == NKT - 1
                vcol = [(2 * half) * 33, (2 * half + 1) * 33]
                # combo0 -> psum rows 0:33, combo1 -> rows 64:97 (den in 32/96);
                # disjoint column strips run concurrently on the PE
                nc.tensor.matmul(
                    av[0:33, :],
                    v_sb[:, kt, vcol[0] : vcol[0] + 33],
                    es[:, 0:512],
                    start=first,
                    stop=last,
                    tile_position=(0, 0),
                )
                nc.tensor.matmul(
                    av[64:97, :],
                    v_sb[:, kt, vcol[1] : vcol[1] + 33],
                    es[:, 512:1024],
                    start=first,
                    stop=last,
                    tile_position=(0, 64),
                )

            def emit_den_scale(qs, pairing, half):
                av = av_tiles.pop((qs, pairing, half))
                if (qs, pairing) not in so_tiles:
                    so_tiles[(qs, pairing)] = so_pool.tile(
                        [128, QW], bf16, tag="so", name="so"
                    )
                so = so_tiles[(qs, pairing)]
                # so rows [c0_h0 | c0_h1 | c1_h0 | c1_h1]
                # both reciprocals first: the DVE is strict FIFO, so a mul
                # that waits on the gpsimd broadcast must not sit ahead of
                # the second (3.4us) reciprocal
                recs, bcs = [], []
                for p0 in (0, 64):
                    rec = rc_pool.tile([1, QW], f32, tag="rc")
                    nc.vector.reciprocal(rec[:], av[p0 + 32 : p0 + 33, :])
                    recs.append(rec)
                for rec in recs:
                    bc = bc_pool.tile([32, QW], f32, tag="bc")
                    nc.gpsimd.partition_broadcast(bc[:], rec[:], channels=32)
                    bcs.append(bc)
                for p0, dst0, bc in ((0, 0, bcs[0]), (64, 64, bcs[1])):
                    nc.vector.tensor_mul(
                        so[dst0 + 32 * half : dst0 + 32 * half + 32, :],
                        av[p0 : p0 + 32, :],
                        bc[:],
                    )

            def emit_projection(qs):
                so_a = so_tiles.pop((qs, 0))
                so_b = so_tiles.pop((qs, 1))
                for mt in range(QW // 128):
                    tok = qs * QW + mt * 128
                    for wsel, ydram in ((sb_wpr, yr), (sb_wpi, yi)):
                        yps = sc_ps.tile([128, 1024], f32, tag="sc")
                        nc.tensor.matmul(
                            yps[:, 0:256],
                            so_a[:, mt * 128 : (mt + 1) * 128],
                            wsel[:, 0, :],
                            start=True,
                            stop=False,
                        )
                        nc.tensor.matmul(
                            yps[:, 0:256],
                            so_b[:, mt * 128 : (mt + 1) * 128],
                            wsel[:, 1, :],
                            start=False,
                            stop=True,
                        )
                        ysb = y_pool.tile([128, 256], f32, tag="y")
                        nc.vector.tensor_copy(ysb[:], yps[:, 0:256])
                        nc.sync.dma_start(ydram[tok : tok + 128, :], ysb[:])

            # lookahead-2 pipeline so the PE never waits on ACT within an
            # iteration; projections are emitted PROJ_DELAY iterations after
            # their den-scale so the PE-side projection matmuls don't stall
            # on the DVE reciprocal chain (stalls >3.4us re-throttle HAM)
            PROJ_DELAY = 14
            pending_proj = []
            for i in range(-2, len(iters)):
                if i + 2 < len(iters):
                    emit_s_exp(iters[i + 2])
                if i == -2:
                    continue
                if i == -1:
                    # V and kbt land on the PE behind the first S matmuls so
                    # the first exp starts ~18us earlier; V feeds AV(0) which
                    # is emitted after it, kbt only the B pairing (iter 32+)
                    emit_v()
                    emit_proj(kbt, sb_wkb)
                    continue
                while pending_proj and pending_proj[0][0] <= i:
                    emit_projection(pending_proj.pop(0)[1])
                emit_av(iters[i])
                qs, pairing, half, kt = iters[i]
                if kt == NKT - 1:
                    emit_den_scale(qs, pairing, half)
                    if pairing == 1 and half == 1:
                        pending_proj.append((i + PROJ_DELAY, qs))
            while pending_proj:
                emit_projection(pending_proj.pop(0)[1])

    nc.compile()
    return nc


def _wstack_qk(top, bot, order):
    """[640, 32*len(order)] contraction stack: rows 0:256 coeff of xr,
    256:512 coeff of xi, 512+ zero. order entries: (head, kind)."""
    out = np.zeros((KT * 128, 32 * len(order)), np.float32)
    for g, (h, kind) in enumerate(order):
        rows = slice(32 * h, 32 * h + 32)
        if kind == "r":  # real part: xr@Wr^T - xi@Wi^T
            t, bo = top[rows], -bot[rows]
        elif kind == "i":  # imag part: xr@Wi^T + xi@Wr^T
            t, bo = bot[rows], top[rows]
        else:  # 'ni': negated imag
            t, bo = -bot[rows], -top[rows]
        out[0:256, 32 * g : 32 * (g + 1)] = t.T
        out[256:512, 32 * g : 32 * (g + 1)] = bo.T
    return out


def _prep_core(inp, c):
    b, hp = divmod(c, 4)
    h0, h1 = 2 * hp, 2 * hp + 1

    xt = np.zeros((KT * 128, N), np.float32)
    xt[0:256] = inp["x_r"][b].T
    xt[256:512] = inp["x_i"][b].T
    xt[512] = 1.0

    wq = _wstack_qk(inp["Wq_r"], inp["Wq_i"], [(h0, "r"), (h0, "i"), (h1, "r"), (h1, "i")])
    kr_top, kr_bot = inp["Wkv_r"][:INNER], inp["Wkv_i"][:INNER]
    wka = _wstack_qk(kr_top, kr_bot, [(h0, "r"), (h0, "ni"), (h1, "r"), (h1, "ni")])
    wkb = _wstack_qk(kr_top, kr_bot, [(h0, "ni"), (h0, "r"), (h1, "ni"), (h1, "r")])

    vv = _wstack_qk(
        inp["Wkv_r"][INNER:], inp["Wkv_i"][INNER:],
        [(h0, "r"), (h0, "i"), (h1, "r"), (h1, "i")],
    )  # [640, 128]
    wv = np.zeros((KT * 128, 132), np.float32)
    for blk in range(4):
        wv[:, 33 * blk : 33 * blk + 32] = vv[:, 32 * blk : 32 * (blk + 1)]
        wv[512, 33 * blk + 32] = 1.0  # ones column via the ones-row of x_aug

    # projection stacks: rows of scaledO_A = [rr_h0 | rr_h1 | ii_h0 | ii_h1],
    # scaledO_B = [ri_h0 | ri_h1 | ir_h0 | ir_h1]
    wor = inp["Wo_r"]
    woi = inp["Wo_i"]

    def ocols(h):
        return slice(32 * h, 32 * h + 32)

    wpr = np.zeros((2, 128, 256), np.float32)
    wpi = np.zeros((2, 128, 256), np.float32)
    for j, h in ((0, h0), (1, h1)):
        wpr[0, 32 * j : 32 * j + 32] = wor[:, ocols(h)].T
        wpr[0, 64 + 32 * j : 96 + 32 * j] = -wor[:, ocols(h)].T
        wpr[1, 32 * j : 32 * j + 32] = -woi[:, ocols(h)].T
        wpr[1, 64 + 32 * j : 96 + 32 * j] = -woi[:, ocols(h)].T
        wpi[0, 32 * j : 32 * j + 32] = woi[:, ocols(h)].T
        wpi[0, 64 + 32 * j : 96 + 32 * j] = -woi[:, ocols(h)].T
        wpi[1, 32 * j : 32 * j + 32] = wor[:, ocols(h)].T
        wpi[1, 64 + 32 * j : 96 + 32 * j] = wor[:, ocols(h)].T

    return {
        "xt": xt.reshape(KT, 128, N).astype(_BF16),
        "wq": wq.reshape(KT, 128, 128).astype(_BF16),
        "wka": wka.reshape(KT, 128, 128).astype(_BF16),
        "wkb": wkb.reshape(KT, 128, 128).astype(_BF16),
        "wv": wv.reshape(KT, 128, 132).astype(_BF16),
        "wpr": wpr.astype(_BF16),
        "wpi": wpi.astype(_BF16),
    }


TRACE = False
TRACE_DIR = None
LAST_RESULT = None


def kernel(**inputs):
    global LAST_RESULT
    from concourse.bass_utils import run_bass_kernel_spmd

    inputs = {k: np.asarray(v, dtype=np.float32) for k, v in inputs.items()}
    if "nc" not in _cache:
        _cache["nc"] = _build()
    nc = _cache["nc"]

    in_maps = [_prep_core(inputs, c) for c in range(NCORES)]
    kw = {}
    if TRACE:
        kw = {"trace": True, "tmpdir": TRACE_DIR}
    res = run_bass_kernel_spmd(nc, in_maps, list(range(NCORES)), **kw)
    LAST_RESULT = res

    out = np.zeros((B, N, DIM, 2), np.float32)
    for c in range(NCORES):
        b = c // 4
        out[b, :, :, 0] += res.results[c]["yr"]
        out[b, :, :, 1] += res.results[c]["yi"]
    return out
